# revision 29
# baseline (speedup 1.0000x reference)
"""Trainium2 Bass kernel for single-head attention.

Reference computation (per batch b):
    q = x @ Wq; k = x @ Wk; v = x @ Wv            # [T, D]
    S = (q @ k.T) * C**-0.5                        # [T, T]
    P = softmax(S, axis=-1)
    out = P @ v                                    # [T, D]

Shapes: x [16, 4096, 1024] f32, W* [1024, 64] f32, out [16, 4096, 64] f32.

Sharding: data-parallel over batch across 8 cores (2 batches/core), weights
replicated.

Per-core kernel strategy (all on-chip compute in bf16, fp32 accumulation):
 - x is cast to bf16 on host; loaded transposed (xT, C on partitions) via
   DMA-transpose so the contraction dim of the projections is on partitions.
 - QK projection fused: lhsT = [Wq | Wk] -> qkT [128, T] (qT rows 0:64,
   kT rows 64:128); an SBUF->SBUF DMA builds the swapped copy [kT; qT] so
   score matmuls for two k-tiles run CONCURRENTLY on PE row groups 0-63 and
   64-127 (matmul operands must live in the issuing row group's partitions).
 - V projected to vT [64, T], then PE-transposed to natural v tiles
   [128, 65] with a ones column appended (softmax denominator comes out of
   the PV matmul for free).
 - Scores computed transposed: S^T tile [128 k, 512 q] = kT_tile.T @ qT.
   Each step produces TWO single-bank PSUM score tiles (4-bank ring); the
   softmax exp is split across two engines running concurrently: the ACT
   engine computes exact exp on half a, the DVE engine computes a
   Schraudolph fast-exp on half b (one tensor_scalar: round(S*EXPA+EXPB)
   written as int16 whose bits ARE the bf16 of ~exp(S/32); ~2% rms error
   on that half, which softmax normalization mostly cancels — overall rel
   err ~1e-2 vs the 2e-2 gate). Splitting the exp across both engines is
   what breaks the original ACT-only bottleneck (~330us busy/core).
 - PV: acc[65, 512] += [v|1]_tile.T @ P^T accumulated over all 32 k-tiles in
   PSUM. Row 64 of acc is the softmax denominator.
 - Finalize: acc -> SBUF (ACT), 4 PE-transposes into one single-bank PSUM
   tile, one strided batched reciprocal (DVE), per-partition scaling via
   ACT copy-with-scale, DMA out fp32.
 - All PSUM->SBUF copies run on ACT (Copy shares the act table with Exp,
   no table reloads); the DVE queue stays a nearly pure exp stream so its
   FIFO is never head-of-line blocked behind copies waiting on producers.

Engine queues are FIFO, so emission order = per-engine execution order. The
emission is software-pipelined: each batch's projection work is emitted in
single-matmul units interleaved through the previous batch's attention
stream, each PV pair is emitted two steps behind its exp (sem waits are
satisfied at decode time; the PE wait queue is only 4 deep and an
unsatisfied wait head-of-line blocks the sequencer), and under For_i the
projection of the first batch wraps around the loop back edge so the
iteration seam is pipelined too.
"""

import numpy as np
import ml_dtypes

B, T, C, D = 16, 4096, 1024, 64
N_CORES = 8
NB = B // N_CORES  # batches per core
P = 128
KT = T // P  # 32 k-tiles per batch
CT = C // P  # 8 contraction tiles for projections
NQ = 1024  # q-chunk processed per exp/PV step
NQC = T // NQ  # 4 q-chunks per batch
T2 = T // 2  # half-batch T extent for xT staging
SCALE = float(C) ** -0.5

# Softmax exp is split between the ACT engine (exact exp, bf16 out) and the
# DVE engine (Schraudolph fast-exp: one tensor_scalar computing
# round(S*EXPA + EXPB) into int16, whose bits reinterpret as bf16 give
# approximately exp(S*SCALE); ~2% rms per-element error on those tiles,
# which the softmax normalization mostly tolerates). k-tile-pair steps whose
# tp index lands in DVE_TPS take the DVE path; the alternation also
# ping-pongs the two PSUM score banks between the two consumer engines.
import math

EXPA = SCALE * 128.0 / math.log(2.0)
EXPB = 16256.0 - 7.25
DVE_TPS = frozenset((1, 3, 5, 7, 9, 11, 13, 15))

_BF16 = ml_dtypes.bfloat16

_cached_nc = None


def _patch_tile_drain():
    """walrus in this toolchain rejects instructions with >1 sync wait on the
    Drain opcode; split the TileContext exit drain into 1-wait drains."""
    import bass_rust
    import concourse.tile as tile
    from concourse.tile import ScopedClock

    if getattr(tile.TileContext, "_drain_split_patched", False):
        return

    def _split_drain_and_barrier(self, tick_clock, wait_clock):
        drain_inst = self.nc.sync.drain()
        wait_clock.add_sem_waits(
            drain_inst.ins, ScopedClock({None: tick_clock.global_clock})
        )
        si = drain_inst.ins.sync_info
        waits = list(si.on_wait) if si is not None else []
        if len(waits) > 1:
            si.on_wait = waits[:1]
            drain_inst.ins.sync_info = si
            for i in range(1, len(waits)):
                extra = self.nc.sync.drain()
                extra.ins.sync_info = bass_rust.SyncInfo(
                    on_wait=waits[i : i + 1], on_update=[]
                )
        self.nc.all_engine_barrier()
        popped = self.nc._tile_sem_poison_stack.pop()
        assert popped is self._sem_poison
        self.nc.clear_and_free_semaphores(list(self.sems.allocated().values()))
        self.nc.all_engine_barrier()

    tile.TileContext._drain_and_barrier = _split_drain_and_barrier
    tile.TileContext._drain_split_patched = True


def _split_multi_wait_instructions(nc):
    """walrus in this toolchain allows at most one sync wait per instruction.
    Hoist extra waits onto nop instructions inserted immediately before, on
    the same engine (engine queues are FIFO, so ordering is preserved)."""
    import bass_rust

    for func in nc.m.functions:
        for bb in func.blocks:
            insts = list(bb.instructions)
            if not any(
                ins.sync_info is not None and len(ins.sync_info.on_wait) > 1
                for ins in insts
            ):
                continue
            cur_bb = nc.cur_bb.bb if nc.cur_bb is not None else None
            cur_snapshot = list(cur_bb.instructions) if cur_bb is not None else None
            new_list = []
            for ins in insts:
                si = ins.sync_info
                if si is not None and len(si.on_wait) > 1:
                    waits = list(si.on_wait)
                    eng = nc.engines[ins.engine]
                    for w in waits[:-1]:
                        nop = eng.nop(nofuse=True, hint="wait_split")
                        nop.ins.sync_info = bass_rust.SyncInfo(
                            on_wait=[w], on_update=[]
                        )
                        new_list.append(nop.ins)
                    si.on_wait = waits[-1:]
                    ins.sync_info = si
                new_list.append(ins)
            if cur_bb is not None and cur_bb.name != bb.name:
                # nops were appended to cur_bb; strip them from there
                cur_bb.instructions = cur_snapshot
            bb.instructions = new_list


def build_nc(
    repeat=1,
    loop_n=0,
    packed_s=True,
    ablate=None,
    dve_tps=DVE_TPS,
    wrap=True,
    use_dmat=False,
    batch_recip=True,
):
    """Build the per-core Bass program (identical on all 8 cores).

    repeat > 1 re-runs the whole per-core workload that many times (writing
    the same outputs); loop_n > 0 additionally wraps the workload in an
    on-device For_i loop with that many iterations. Both are used by the
    timing harness to separate HW execution time from the (large, noisy)
    dispatch overhead of this environment via the wall-time slope."""
    import concourse.bass as bass
    import concourse.tile as tile
    from concourse import mybir
    from concourse.bass import ds, ts
    from concourse.masks import make_identity

    _patch_tile_drain()

    f32 = mybir.dt.float32
    bf16 = mybir.dt.bfloat16

    nc = bass.Bass()
    x_in = nc.dram_tensor("x", [NB, T, C], bf16, kind="ExternalInput")
    wqk_in = nc.dram_tensor("wqk", [C, 2 * D], bf16, kind="ExternalInput")
    wv_in = nc.dram_tensor("wv", [C, D], bf16, kind="ExternalInput")
    out_dram = nc.dram_tensor("out", [NB, T, D], f32, kind="ExternalOutput")

    with tile.TileContext(nc) as tc:
        with (
            tc.tile_pool(name="weights", bufs=1) as wpool,
            tc.tile_pool(name="xT", bufs=2) as xpool,
            tc.tile_pool(name="qk", bufs=2) as qkpool,
            tc.tile_pool(name="kT", bufs=2) as ktpool,
            tc.tile_pool(name="vT", bufs=2) as vtpool,
            tc.tile_pool(name="vs", bufs=2) as vspool,
            tc.tile_pool(name="pt", bufs=6) as ptpool,
            tc.tile_pool(name="oT", bufs=2) as otpool,
            tc.tile_pool(name="r", bufs=4) as rpool,
            tc.tile_pool(name="osb", bufs=3) as opool,
            tc.tile_pool(name="psbig", bufs=4, space="PSUM") as psbig,
            tc.tile_pool(name="psacc", bufs=2, space="PSUM") as psacc,
            tc.tile_pool(name="pstmp", bufs=2, space="PSUM") as pstmp,
        ):
            # --- constants ---
            wqk_sb = wpool.tile([P, CT, 2 * D], bf16)
            nc.sync.dma_start(
                wqk_sb[:], wqk_in.rearrange("(c p) m -> p c m", p=P)
            )
            wv_sb = wpool.tile([P, CT, D], bf16)
            nc.sync.dma_start(wv_sb[:], wv_in.rearrange("(c p) m -> p c m", p=P))
            ident_bf = wpool.tile([P, P], bf16)
            make_identity(nc, ident_bf[:])
            ident_f32 = wpool.tile([P, P], f32)
            make_identity(nc, ident_f32[:])
            if ablate == "noact":
                pt_const = wpool.tile([P, 1024], bf16)
                nc.gpsimd.memset(pt_const[:], 0.0078)
            sink = wpool.tile([P, 1], f32)
            nc.gpsimd.memset(sink[:], 0.0)

            import contextlib

            # Per-batch state (tiles shared between proj- and attn-units)
            state = {}

            def proj_units(b):
                """Projection pipeline for batch b as a list of emission
                units. Interleaved into the previous batch's attention
                stream so the (FIFO) PE queue has projection work filling
                the slack of the ACT-paced attention groups."""
                units = []

                def load(h, b=b):
                    xt = xpool.tile([P, CT, T2], bf16, name=f"xh_{b}_{h}", tag="xh")
                    if ablate == "fastx":
                        # timing probe: same byte volume, contiguous (wrong
                        # layout, breaks correctness)
                        nc.sync.dma_start(
                            out=xt[:],
                            in_=x_in[b].rearrange("(p q) c -> p (q c)", p=P)[
                                :, ds(h * CT * T2, CT * T2)
                            ],
                        )
                    else:
                        for c in range(CT):
                            nc.sync.dma_start(
                                out=xt[:, c, :],
                                in_=x_in[b, h * T2 : (h + 1) * T2, ts(c, P)],
                                transpose=True,
                            )
                    state[(b, "xh", h)] = xt

                units.append(lambda h=0: load(h))
                units.append(lambda h=1: load(h))

                def qk_mm(n, c, b=b):
                    if (b, "qkT") not in state:
                        state[(b, "qkT")] = qkpool.tile(
                            [P, T], bf16, name=f"qkT_{b}", tag="qkT"
                        )
                    if c == 0:
                        state[(b, "pps")] = pstmp.tile(
                            [P, 512], f32, name=f"pps_{b}_{n}", tag="tmp"
                        )
                    ps = state[(b, "pps")]
                    xt = state[(b, "xh", n // (T2 // 512))]
                    off = (n % (T2 // 512)) * 512
                    nc.tensor.matmul(
                        ps[:],
                        wqk_sb[:, c, :],
                        xt[:, c, ds(off, 512)],
                        start=(c == 0),
                        stop=(c == CT - 1),
                    )
                    if c == CT - 1:
                        nc.scalar.copy(
                            state[(b, "qkT")][:, ts(n, 512)], ps[:]
                        )

                for n in range(T // 512):
                    for c in range(CT):
                        units.append(lambda n=n, c=c: qk_mm(n, c))

                def qk_swap(b=b):
                    # swapped copy [kT; qT] so both PE row groups can run
                    # score matmuls concurrently (operands must live in the
                    # issuing row group's partition range)
                    qkT = state[(b, "qkT")]
                    if (b, "qkT2") not in state:
                        state[(b, "qkT2")] = ktpool.tile(
                            [P, T], bf16, name=f"qkT2_{b}", tag="qkT2"
                        )
                    qkT2 = state[(b, "qkT2")]
                    nc.sync.dma_start(qkT2[0:64, :], qkT[64:128, :])
                    nc.sync.dma_start(qkT2[64:128, :], qkT[0:64, :])

                units.append(qk_swap)

                def v_mm(n, c, b=b):
                    if (b, "vT") not in state:
                        state[(b, "vT")] = vtpool.tile(
                            [64, T], bf16, name=f"vT_{b}", tag="vT"
                        )
                    if c == 0:
                        state[(b, "vps")] = pstmp.tile(
                            [64, 512], f32, name=f"vps_{b}_{n}", tag="tmp"
                        )
                    ps = state[(b, "vps")]
                    xt = state[(b, "xh", n // (T2 // 512))]
                    off = (n % (T2 // 512)) * 512
                    nc.tensor.matmul(
                        ps[:],
                        wv_sb[:, c, :],
                        xt[:, c, ds(off, 512)],
                        start=(c == 0),
                        stop=(c == CT - 1),
                    )
                    if c == CT - 1:
                        nc.scalar.copy(
                            state[(b, "vT")][:, ts(n, 512)], ps[:]
                        )

                for n in range(T // 512):
                    for c in range(CT):
                        units.append(lambda n=n, c=c: v_mm(n, c))

                def v_nat(g, b=b):
                    # 2 k-tiles of vT into natural layout via SBUF->SBUF
                    # DMA transpose (no PE transposes / ACT copies needed)
                    if (b, "vsb") not in state:
                        vsb = vspool.tile([P, KT, D + 1], bf16, name=f"vsb_{b}", tag="vsb")
                        nc.gpsimd.memset(vsb[:, :, D], 1.0)
                        state[(b, "vsb")] = vsb
                    vsb = state[(b, "vsb")]
                    vT = state[(b, "vT")]
                    for t in range(g * 2, (g + 1) * 2):
                        if use_dmat:
                            nc.sync.dma_start(
                                out=vsb[:, t, 0:D],
                                in_=vT[:, ts(t, P)],
                                transpose=True,
                            )
                        else:
                            pst = pstmp.tile([P, D], bf16, tag="tmp")
                            nc.tensor.transpose(
                                pst[:], vT[:, ts(t, P)], ident_bf[0:64, 0:64]
                            )
                            nc.scalar.copy(vsb[:, t, 0:D], pst[:])

                for g in range(KT // 2):
                    units.append(lambda g=g: v_nat(g))
                return units

            def attn_units(b):
                """Attention + finalize for batch b as emission units:
                one unit per (nq, k-tile-pair) plus one finalize unit per
                nq chunk."""
                units = []

                def emit_pv(nq, tp, pt, b=b):
                    acc = state[(b, "acc", nq)]
                    vsb = state[(b, "vsb")]
                    nc.tensor.matmul(
                        acc[:],
                        vsb[:, 2 * tp, :],
                        pt[:, 0:512],
                        start=(tp == 0),
                        stop=False,
                    )
                    nc.tensor.matmul(
                        acc[:],
                        vsb[:, 2 * tp + 1, :],
                        pt[:, 512:1024],
                        start=False,
                        stop=(tp == KT // 2 - 1),
                    )

                def attn_step(nq, tp, b=b):
                    # emits S-pair(tp) and exp(tp); the PV of step tp-1 is
                    # emitted after S(tp) so the PE queue never head-of-line
                    # blocks on the exp of the current step.
                    qkT = state[(b, "qkT")]
                    qkT2 = state[(b, "qkT2")]
                    if (b, "oT") not in state:
                        state[(b, "oT")] = otpool.tile([65, T], f32, name=f"oT_{b}", tag="oT")
                    if tp == 0:
                        state[(b, "acc", nq)] = psacc.tile(
                            [65, 512], f32, name=f"acc_{b}_{nq}", tag="acc"
                        )
                    # two single-bank S tiles per step in a 4-bank ring: the
                    # WAR wait for a bank is one half-width exp, not a full
                    # 1024-wide one, so the PE never stalls long on psbig
                    s_a = psbig.tile([P, 512], f32, tag="big", name="s_a")
                    s_b = psbig.tile([P, 512], f32, tag="big", name="s_b")
                    nc.tensor.matmul(
                        s_a[:],
                        qkT2[0:64, ts(2 * tp, P)],
                        qkT[0:64, ds(nq * 512, 512)],
                        start=True,
                        stop=True,
                    )
                    if ablate == "nopair":
                        # timing probe: same math, but both score matmuls on
                        # PE row group 0-63 so they serialize
                        nc.tensor.matmul(
                            s_b[:],
                            qkT2[0:64, ts(2 * tp + 1, P)],
                            qkT[0:64, ds(nq * 512, 512)],
                            start=True,
                            stop=True,
                        )
                    else:
                        nc.tensor.matmul(
                            s_b[:],
                            qkT[64:128, ts(2 * tp + 1, P)],
                            qkT2[64:128, ds(nq * 512, 512)],
                            start=True,
                            stop=True,
                        )
                    # emit the PV of step tp-2: two steps of lead so its
                    # sem wait on the exp engines is satisfied at decode
                    # time (the PE wait queue is only 4 deep; an
                    # unsatisfied wait head-of-line blocks the sequencer)
                    pend = state.setdefault((b, "pv"), [])
                    if len(pend) >= 2:
                        emit_pv(*pend.pop(0))
                    if ablate == "noact":
                        nc.vector.tensor_copy(sink[0:1, 0:1], s_a[0:1, 0:1])
                        pt = pt_const
                    else:
                        # fixed half-assignment: ACT exps half a (exact exp),
                        # DVE half b (Schraudolph), into disjoint halves of
                        # one pt tile
                        pt = ptpool.tile([P, 1024], bf16)
                        nc.scalar.activation(
                            pt[:, 0:512],
                            s_a[:],
                            mybir.ActivationFunctionType.Exp,
                            scale=SCALE,
                        )
                        if dve_tps:
                            nc.vector.tensor_scalar(
                                pt[:, 512:1024].bitcast(mybir.dt.int16),
                                s_b[:],
                                EXPA,
                                EXPB,
                                op0=mybir.AluOpType.mult,
                                op1=mybir.AluOpType.add,
                            )
                        else:
                            nc.scalar.activation(
                                pt[:, 512:1024],
                                s_b[:],
                                mybir.ActivationFunctionType.Exp,
                                scale=SCALE,
                            )
                    if ablate == "nopv":
                        nc.vector.tensor_copy(sink[0:1, 0:1], pt[0:1, 0:1])
                        return
                    pend.append((nq, tp, pt))

                def finalize_a(nq, b=b):
                    # acc -> SBUF (ACT), then the 4 per-128-q transposes into
                    # one single-bank PSUM tile (disjoint column slices)
                    if ablate == "nopv":
                        return
                    for pending in state.get((b, "pv"), []):
                        emit_pv(*pending)
                    state[(b, "pv")] = []
                    oT = state[(b, "oT")]
                    acc = state.pop((b, "acc", nq))
                    nc.scalar.copy(oT[:, ds(nq * 512, 512)], acc[:])
                    pso = pstmp.tile([P, 4 * 65], f32, tag="tmp")
                    for j in range(4):
                        m = nq * 4 + j
                        nc.tensor.transpose(
                            pso[:, ds(j * 65, 65)],
                            oT[:, ds(m * P, P)],
                            ident_f32[0:65, 0:65],
                        )
                    state[(b, "pso", nq)] = pso

                def finalize_b(nq, b=b):
                    # reciprocal of the denominators (DVE, tiny) and the
                    # final per-partition scaling (ACT copy-with-scale)
                    if ablate == "nopv":
                        return
                    pso = state.pop((b, "pso", nq))
                    osb = opool.tile([P, 4, D], f32)
                    r = rpool.tile([P, 4], f32)
                    if batch_recip:
                        nc.vector.reciprocal(r[:], pso[:, 64 : 4 * 65 : 65])
                    else:
                        for j in range(4):
                            nc.vector.reciprocal(
                                r[:, j : j + 1],
                                pso[:, j * 65 + 64 : j * 65 + 65],
                            )
                    for j in range(4):
                        nc.scalar.mul(
                            osb[:, j, :], pso[:, ds(j * 65, D)], r[:, j : j + 1]
                        )
                    nc.sync.dma_start(
                        out=out_dram[b].rearrange("(n p) d -> p n d", p=P)[
                            :, ds(nq * 4, 4), :
                        ],
                        in_=osb[:],
                    )

                pend_a, pend_b = [], []
                for nq in range(T // 512):
                    for tp in range(KT // 2):
                        units.append(lambda nq=nq, tp=tp: attn_step(nq, tp))
                        if tp == 1 and pend_a:
                            units.append(pend_a.pop())
                        if tp == 3 and pend_b:
                            units.append(pend_b.pop())
                    pend_a.append(lambda nq=nq: finalize_a(nq))
                    pend_b.append(lambda nq=nq: finalize_b(nq))
                units.extend(pend_a)
                units.extend(pend_b)
                return units

            def emit_interleaved(attn, proj):
                """Emit attention units with projection units of the next
                batch spread evenly through them (PE queues are FIFO, so
                emission order is execution order per engine)."""
                if not proj:
                    for u in attn:
                        u()
                    return
                ratio = len(attn) / len(proj)
                pi = 0
                for i, u in enumerate(attn):
                    u()
                    while pi < len(proj) and (pi + 1) * ratio <= i + 1:
                        proj[pi]()
                        pi += 1
                while pi < len(proj):
                    proj[pi]()
                    pi += 1

            batches = [b for _ in range(repeat) for b in range(NB)]
            if ablate == "projonly":
                loop_ctx = (
                    tc.For_i(0, loop_n, 1) if loop_n else contextlib.nullcontext()
                )
                with loop_ctx:
                    for b in batches:
                        for u in proj_units(b):
                            u()
            elif loop_n and wrap:
                # Software-pipelined across the For_i back edge: the
                # prologue projection of batch 0 runs once before the loop;
                # inside the body each batch's attention carries the NEXT
                # batch's projection, wrapping around so the last batch
                # hides the first batch's projection for the next
                # iteration. proj_units reuses the same static tiles via
                # `state`, so the back-edge RAW/WAR deps are the ordinary
                # tile-pool semaphores.
                for u in proj_units(batches[0]):
                    u()
                with tc.For_i(0, loop_n, 1):
                    for i, b in enumerate(batches):
                        nxt = proj_units(batches[(i + 1) % len(batches)])
                        emit_interleaved(attn_units(b), nxt)
            else:
                loop_ctx = (
                    tc.For_i(0, loop_n, 1) if loop_n else contextlib.nullcontext()
                )
                with loop_ctx:
                    for u in proj_units(batches[0]):
                        u()
                    for i, b in enumerate(batches):
                        nxt = (
                            proj_units(batches[i + 1])
                            if i + 1 < len(batches)
                            else []
                        )
                        emit_interleaved(attn_units(b), nxt)

            if ablate in ("nopv", "projonly"):
                dummy = wpool.tile([P, D], f32)
                nc.gpsimd.memset(dummy[:], 0.0)
                for b_ in range(NB):
                    nc.sync.dma_start(
                        out_dram[b_, 0:P, :], dummy[:]
                    )

    _split_multi_wait_instructions(nc)
    return nc


def _get_nc():
    global _cached_nc
    if _cached_nc is None:
        _cached_nc = build_nc()
    return _cached_nc


def make_in_maps(x, Wq, Wk, Wv):
    """Host-side prep: cast to bf16, fuse Wq|Wk, shard batch across cores."""
    xb = np.ascontiguousarray(x).astype(_BF16)
    wqk = np.concatenate([Wq, Wk], axis=1).astype(_BF16)
    wv = np.ascontiguousarray(Wv).astype(_BF16)
    return [
        {"x": xb[i * NB : (i + 1) * NB], "wqk": wqk, "wv": wv}
        for i in range(N_CORES)
    ]


def kernel(x, Wq, Wk, Wv):
    from concourse.bass_utils import run_bass_kernel_spmd

    nc = _get_nc()
    in_maps = make_in_maps(x, Wq, Wk, Wv)
    res = run_bass_kernel_spmd(nc, in_maps, list(range(N_CORES)))
    return np.concatenate(
        [res.results[i]["out"] for i in range(N_CORES)], axis=0
    ).astype(np.float32)

